# revision 4
# baseline (speedup 1.0000x reference)
"""EnergyNet score kernel for 8 TRN2 NeuronCores.

forward(): score_x = d/dx [ -||s|| + sum(s) + c ],  s = MLP(concat(x, t)).
Data-parallel over the batch axis. Per core (Nc samples), features-major:
  a1 = W1^T xaug            (PE, bf16)         h1 = gelu(a1), ga1 = gelu'(a1)  (ACT)
  a2 = W2^T h1              (PE)               h2, ga2                         (ACT)
  s  = h2-stationary @ W3   (PE, sample-major [128s, 64f] PSUM)
  q  = sum(s^2)  -> negr = -1/sqrt(q)          (DVE square/reduce + bit-trick rsqrt)
  srT = s * negr  (= -s*r, sample-major)       (DVE tensor_scalar per-partition scalar)
  sr  = transpose(srT)  [64f, Ns]              (PE transpose + DVE copy)
  u  = W3 @ sr   (= -W3 (s r))                 (PE)
  da2 = (u + rowsum(W3)) * ga2                 (DVE scalar_tensor_tensor; dh2 = W3@(1-s r))
  dh1 = W2 @ da2                               (PE)
  da1 = dh1 * ga1                              (DVE)
  dxT = da1-stationary @ W1x^T  [128s, 64f]    (PE)  -> dx out                 (ACT copy)
"""

import numpy as np
import ml_dtypes
from contextlib import ExitStack

import concourse.bass as bass
import concourse.mybir as mybir
import concourse.tile as tile
from concourse import bacc
from concourse.bass_utils import run_bass_kernel_spmd

dt = mybir.dt
AF = mybir.ActivationFunctionType
ALU = mybir.AluOpType

D = 64          # data dim
DA = 65         # D + 1 (concat t)
H = 256         # hidden
NCORES = 8
NS = 2048       # samples per mega-tile
NT = 512        # samples per matmul subtile
BF = ml_dtypes.bfloat16
MAGIC = 0x5F3759DF


def build(nc_samples: int, with_b3: bool):
    n_megas = nc_samples // NS
    assert n_megas * NS == nc_samples
    nc = bacc.Bacc("TRN2", target_bir_lowering=False)

    xT = nc.declare_dram_parameter("xT", [DA, nc_samples], dt.bfloat16, isOutput=False)
    w1 = nc.declare_dram_parameter("w1", [DA, H], dt.bfloat16, isOutput=False)
    w2 = nc.declare_dram_parameter("w2", [H, H], dt.bfloat16, isOutput=False)      # 2 x [128, 256] k-chunks stacked
    w2t = nc.declare_dram_parameter("w2t", [H, H], dt.bfloat16, isOutput=False)    # W2.T, same chunking
    w3 = nc.declare_dram_parameter("w3", [H, D], dt.bfloat16, isOutput=False)      # 2 x [128, 64]
    w3t = nc.declare_dram_parameter("w3t", [D, H], dt.bfloat16, isOutput=False)    # [64, 256]
    w1xt = nc.declare_dram_parameter("w1xt", [H, D], dt.bfloat16, isOutput=False)  # W1[:64].T, 2 x [128, 64]
    b1p = nc.declare_dram_parameter("b1p", [128, 2], dt.float32, isOutput=False)
    b2p = nc.declare_dram_parameter("b2p", [128, 2], dt.float32, isOutput=False)
    w3s = nc.declare_dram_parameter("w3s", [128, 2], dt.float32, isOutput=False)   # rowsum(W3) chunks
    idn = nc.declare_dram_parameter("idn", [128, 128], dt.bfloat16, isOutput=False)
    if with_b3:
        b3bc = nc.declare_dram_parameter("b3bc", [128, NS // 128 * D], dt.float32, isOutput=False)
    dx = nc.declare_dram_parameter("dx", [nc_samples, D], dt.float32, isOutput=True)
    # view: sample n = chunk*128 + p  ->  [p, chunk, f]
    dx_v = dx.rearrange("(c p) f -> p c f", p=128)

    with tile.TileContext(nc) as tc, ExitStack() as ctx:
        cst = ctx.enter_context(tc.tile_pool(name="cst", bufs=1))
        xp = ctx.enter_context(tc.tile_pool(name="xp", bufs=2))
        h1p = ctx.enter_context(tc.tile_pool(name="h1p", bufs=3))
        h2p = ctx.enter_context(tc.tile_pool(name="h2p", bufs=3))
        g1p = ctx.enter_context(tc.tile_pool(name="g1p", bufs=3))
        g2p = ctx.enter_context(tc.tile_pool(name="g2p", bufs=3))
        dap = ctx.enter_context(tc.tile_pool(name="dap", bufs=3))
        nrm = ctx.enter_context(tc.tile_pool(name="nrm", bufs=2))
        sml = ctx.enter_context(tc.tile_pool(name="sml", bufs=2))
        dxp = ctx.enter_context(tc.tile_pool(name="dxp", bufs=2))
        fps = ctx.enter_context(tc.tile_pool(name="fps", bufs=2, space="PSUM"))
        sps = ctx.enter_context(tc.tile_pool(name="sps", bufs=1, space="PSUM"))
        tps = ctx.enter_context(tc.tile_pool(name="tps", bufs=1, space="PSUM"))
        bps = ctx.enter_context(tc.tile_pool(name="bps", bufs=2, space="PSUM"))
        ops = ctx.enter_context(tc.tile_pool(name="ops", bufs=1, space="PSUM"))

        # ---- one-time constants ----
        w1_s = cst.tile([DA, H], dt.bfloat16, name="w1_s")
        nc.sync.dma_start(w1_s[:], w1[:])
        w2_s = [cst.tile([128, H], dt.bfloat16, name=f"w2_{k}") for k in range(2)]
        w2t_s = [cst.tile([128, H], dt.bfloat16, name=f"w2t_{k}") for k in range(2)]
        w3_s = [cst.tile([128, D], dt.bfloat16, name=f"w3_{k}") for k in range(2)]
        w1xt_s = [cst.tile([128, D], dt.bfloat16, name=f"w1xt_{k}") for k in range(2)]
        for k in range(2):
            nc.sync.dma_start(w2_s[k][:], w2[128 * k:128 * (k + 1), :])
            nc.sync.dma_start(w2t_s[k][:], w2t[128 * k:128 * (k + 1), :])
            nc.sync.dma_start(w3_s[k][:], w3[128 * k:128 * (k + 1), :])
            nc.sync.dma_start(w1xt_s[k][:], w1xt[128 * k:128 * (k + 1), :])
        w3t_s = cst.tile([D, H], dt.bfloat16, name="w3t_s")
        nc.sync.dma_start(w3t_s[:], w3t[:])
        b1_s = cst.tile([128, 2], dt.float32, name="b1_s")
        nc.sync.dma_start(b1_s[:], b1p[:])
        b2_s = cst.tile([128, 2], dt.float32, name="b2_s")
        nc.sync.dma_start(b2_s[:], b2p[:])
        w3s_s = cst.tile([128, 2], dt.float32, name="w3s_s")
        nc.sync.dma_start(w3s_s[:], w3s[:])
        idn_s = cst.tile([128, 128], dt.bfloat16, name="idn_s")
        nc.sync.dma_start(idn_s[:], idn[:])
        magic_s = cst.tile([128, NS // 128], dt.int32, name="magic_s")
        nc.gpsimd.memset(magic_s[:], MAGIC)
        if with_b3:
            b3bc_s = cst.tile([128, NS // 128 * D], dt.float32, name="b3bc_s")
            nc.sync.dma_start(b3bc_s[:], b3bc[:])

        NCH = NS // 128  # 128-sample chunks per mega (16)

        for mg in range(n_megas):
            x_t = xp.tile([DA, NS], dt.bfloat16, tag="x")
            nc.sync.dma_start(x_t[:], xT[:, mg * NS:(mg + 1) * NS])

            h1_t = [h1p.tile([128, NS], dt.bfloat16, tag=f"h1_{m}", name=f"h1_{m}") for m in range(2)]
            g1_t = [g1p.tile([128, NS], dt.bfloat16, tag=f"g1_{m}", name=f"g1_{m}") for m in range(2)]
            h2_t = [h2p.tile([128, NS], dt.bfloat16, tag=f"h2_{m}", name=f"h2_{m}") for m in range(2)]
            g2_t = [g2p.tile([128, NS], dt.bfloat16, tag=f"g2_{m}", name=f"g2_{m}") for m in range(2)]
            s_ps = sps.tile([128, NCH * D], dt.float32, tag="s")

            # ---------------- forward ----------------
            for j in range(NS // NT):
                sl = slice(j * NT, (j + 1) * NT)
                for m in range(2):
                    a1 = fps.tile([128, NT], dt.float32, tag="fwd")
                    nc.tensor.matmul(a1[:], w1_s[:, 128 * m:128 * (m + 1)], x_t[:, sl],
                                     start=True, stop=True)
                    nc.scalar.activation(h1_t[m][:, sl], a1[:], AF.Gelu,
                                         bias=b1_s[:, m:m + 1])
                    nc.scalar.activation(g1_t[m][:, sl], a1[:], AF.Derivative_Gelu,
                                         bias=b1_s[:, m:m + 1])
                for m in range(2):
                    a2 = fps.tile([128, NT], dt.float32, tag="fwd")
                    for k in range(2):
                        nc.tensor.matmul(a2[:], w2_s[k][:, 128 * m:128 * (m + 1)],
                                         h1_t[k][:, sl], start=(k == 0), stop=(k == 1))
                    nc.scalar.activation(h2_t[m][:, sl], a2[:], AF.Gelu,
                                         bias=b2_s[:, m:m + 1])
                    nc.scalar.activation(g2_t[m][:, sl], a2[:], AF.Derivative_Gelu,
                                         bias=b2_s[:, m:m + 1])
                # s (sample-major): stationary = h2 128-sample slice, moving = W3 chunk
                for sc in range(4):
                    ch = j * 4 + sc
                    csl = slice(j * NT + sc * 128, j * NT + (sc + 1) * 128)
                    for k in range(2):
                        nc.tensor.matmul(s_ps[:, ch * D:(ch + 1) * D],
                                         h2_t[k][:, csl], w3_s[k][:],
                                         start=(k == 0), stop=(k == 1))

            # ---------------- norm: negr = -1/||s|| ----------------
            if with_b3:
                s_in = nrm.tile([128, NCH * D], dt.float32, tag="sb3")
                nc.vector.tensor_tensor(s_in[:], s_ps[:], b3bc_s[:], ALU.add)
            else:
                s_in = s_ps
            sq = nrm.tile([128, NCH * D], dt.float32, tag="sq")
            nc.scalar.activation(sq[:], s_in[:], AF.Square)
            q = sml.tile([128, NCH], dt.float32, tag="q")
            nc.vector.tensor_reduce(q[:], sq[:].rearrange("p (c f) -> p c f", f=D),
                                    mybir.AxisListType.X, ALU.add)
            # negr via fast-inverse-sqrt + 3 sign-flipping Newton steps
            yi = sml.tile([128, NCH], dt.int32, tag="yi")
            nc.vector.tensor_scalar(yi[:], q[:].bitcast(dt.int32), 1, None,
                                    ALU.logical_shift_right)
            nc.vector.tensor_tensor(yi[:], magic_s[:], yi[:], ALU.subtract)
            y = yi[:].bitcast(dt.float32)
            qh = sml.tile([128, NCH], dt.float32, tag="qh")
            nc.vector.tensor_scalar(qh[:], q[:], 0.5, None, ALU.mult)
            pp = sml.tile([128, NCH], dt.float32, tag="pp")
            for it in range(3):
                nc.vector.tensor_tensor(pp[:], y, y, ALU.mult)
                nc.vector.tensor_tensor(pp[:], pp[:], qh[:], ALU.mult)
                nc.vector.tensor_scalar(pp[:], pp[:], -1.5, None, ALU.add)
                nc.vector.tensor_tensor(y, y, pp[:], ALU.mult)   # y <- -y(1.5-0.5qy^2)
            # broadcast negr along feature axis (stride-0 free dim) on gpsimd
            nrb = nrm.tile([128, NCH * D], dt.float32, tag="nrb")
            nc.gpsimd.tensor_copy(
                nrb[:].rearrange("p (c f) -> p c f", f=D),
                y.broadcast_to([128, NCH, D]))
            # srT = s * (-r)  (sample-major)
            srt = nrm.tile([128, NCH * D], dt.bfloat16, tag="srt")
            nc.vector.tensor_tensor(srt[:], s_in[:], nrb[:], ALU.mult)

            # transpose srT -> sr [64, NS]
            sr = nrm.tile([D, NS], dt.bfloat16, tag="sr")
            for hh in range(2):
                sr_ps = tps.tile([D, NS // 2], dt.bfloat16, tag="srps")
                for c in range(NCH // 2):
                    ch = hh * (NCH // 2) + c
                    nc.tensor.transpose(sr_ps[:, c * 128:(c + 1) * 128],
                                        srt[:, ch * D:(ch + 1) * D], idn_s[:])
                nc.vector.tensor_copy(sr[:, hh * (NS // 2):(hh + 1) * (NS // 2)], sr_ps[:])

            # ---------------- backward ----------------
            da2_t = [dap.tile([128, NS], dt.bfloat16, tag=f"da2_{m}", name=f"da2_{m}") for m in range(2)]
            da1_t = [dap.tile([128, NS], dt.bfloat16, tag=f"da1_{m}", name=f"da1_{m}") for m in range(2)]
            for j in range(NS // NT):
                sl = slice(j * NT, (j + 1) * NT)
                for m in range(2):
                    u = bps.tile([128, NT], dt.float32, tag="bwd")
                    nc.tensor.matmul(u[:], w3t_s[:, 128 * m:128 * (m + 1)], sr[:, sl],
                                     start=True, stop=True)
                    # da2 = (u + rowsum(W3)) * ga2
                    nc.vector.scalar_tensor_tensor(da2_t[m][:, sl], u[:],
                                                   w3s_s[:, m:m + 1], g2_t[m][:, sl],
                                                   ALU.add, ALU.mult)
                for m in range(2):
                    dh1 = bps.tile([128, NT], dt.float32, tag="bwd")
                    for k in range(2):
                        nc.tensor.matmul(dh1[:], w2t_s[k][:, 128 * m:128 * (m + 1)],
                                         da2_t[k][:, sl], start=(k == 0), stop=(k == 1))
                    nc.vector.tensor_tensor(da1_t[m][:, sl], dh1[:], g1_t[m][:, sl],
                                            ALU.mult)
                # dxT (sample-major out): stationary = da1 slice, moving = W1x^T chunk
                dxq = ops.tile([128, 4 * D], dt.float32, tag="dxq")
                for sc in range(4):
                    csl = slice(j * NT + sc * 128, j * NT + (sc + 1) * 128)
                    for k in range(2):
                        nc.tensor.matmul(dxq[:, sc * D:(sc + 1) * D],
                                         da1_t[k][:, csl], w1xt_s[k][:],
                                         start=(k == 0), stop=(k == 1))
                dxs = dxp.tile([128, 4 * D], dt.float32, tag="dxs")
                nc.scalar.copy(dxs[:], dxq[:])
                nc.gpsimd.dma_start(
                    dx_v[:, mg * NCH + j * 4:mg * NCH + (j + 1) * 4, :],
                    dxs[:].rearrange("p (c f) -> p c f", f=D))

    nc.compile()
    return nc


_CACHE = {}


def _get_nc(nc_samples, with_b3):
    key = (nc_samples, with_b3)
    if key not in _CACHE:
        _CACHE[key] = build(nc_samples, with_b3)
    return _CACHE[key]


def kernel(t, x, W1, b1, W2, b2, W3, b3, c):
    t = np.asarray(t); x = np.asarray(x)
    W1 = np.asarray(W1, np.float32); b1 = np.asarray(b1, np.float32)
    W2 = np.asarray(W2, np.float32); b2 = np.asarray(b2, np.float32)
    W3 = np.asarray(W3, np.float32); b3 = np.asarray(b3, np.float32)
    N = t.shape[0]
    npc = N // NCORES
    with_b3 = bool(np.any(b3))
    nc = _get_nc(npc, with_b3)

    xT = np.empty((DA, N), dtype=BF)
    xT[:D] = x.T
    xT[D] = t
    base = dict(
        w1=W1.astype(BF),
        w2=W2.astype(BF),
        w2t=np.ascontiguousarray(W2.T).astype(BF),
        w3=W3.astype(BF),
        w3t=np.ascontiguousarray(W3.T).astype(BF),
        w1xt=np.ascontiguousarray(W1[:D].T).astype(BF),
        b1p=np.ascontiguousarray(b1.reshape(2, 128).T),
        b2p=np.ascontiguousarray(b2.reshape(2, 128).T),
        w3s=np.ascontiguousarray(W3.sum(1).astype(np.float32).reshape(2, 128).T),
        idn=np.eye(128, dtype=BF),
    )
    if with_b3:
        base["b3bc"] = np.tile(b3, (128, NS // 128)).astype(np.float32)
    in_maps = []
    for cid in range(NCORES):
        m = dict(base)
        m["xT"] = np.ascontiguousarray(xT[:, cid * npc:(cid + 1) * npc])
        in_maps.append(m)
    res = run_bass_kernel_spmd(nc, in_maps, list(range(NCORES)))
    return np.concatenate([res.results[i]["dx"] for i in range(NCORES)], axis=0)


# revision 6
# speedup vs baseline: 31.1700x; 31.1700x over previous
"""EnergyNet score kernel for 8 TRN2 NeuronCores.

forward(): score_x = d/dx [ -||s|| + sum(s) + c ],  s = MLP(concat(x, t)).
Data-parallel over the batch axis. Per core (Nc samples), features-major:
  a1 = W1^T xaug            (PE, bf16)         h1 = gelu(a1), ga1 = gelu'(a1)  (ACT)
  a2 = W2^T h1              (PE)               h2, ga2                         (ACT)
  s  = h2-stationary @ W3   (PE, sample-major [128s, 64f] PSUM)
  q  = sum(s^2)  -> negr = -1/sqrt(q)          (DVE square/reduce + bit-trick rsqrt)
  srT = s * negr  (= -s*r, sample-major)       (DVE tensor_scalar per-partition scalar)
  sr  = transpose(srT)  [64f, Ns]              (PE transpose + DVE copy)
  u  = W3 @ sr   (= -W3 (s r))                 (PE)
  da2 = (u + rowsum(W3)) * ga2                 (DVE scalar_tensor_tensor; dh2 = W3@(1-s r))
  dh1 = W2 @ da2                               (PE)
  da1 = dh1 * ga1                              (DVE)
  dxT = da1-stationary @ W1x^T  [128s, 64f]    (PE)  -> dx out                 (ACT copy)
"""

import numpy as np
import ml_dtypes
from contextlib import ExitStack

import concourse.bass as bass
import concourse.mybir as mybir
import concourse.tile as tile
from concourse import bacc
from concourse.bass_utils import run_bass_kernel_spmd

dt = mybir.dt
AF = mybir.ActivationFunctionType
ALU = mybir.AluOpType

D = 64          # data dim
DA = 65         # D + 1 (concat t)
H = 256         # hidden
NCORES = 8
NS = 2048       # samples per mega-tile
NT = 512        # samples per matmul subtile
BF = ml_dtypes.bfloat16
MAGIC = 0x5F3759DF


def build(nc_samples: int, with_b3: bool):
    n_megas = nc_samples // NS
    assert n_megas * NS == nc_samples
    nc = bacc.Bacc("TRN2", target_bir_lowering=False)

    xT = nc.declare_dram_parameter("xT", [DA, nc_samples], dt.bfloat16, isOutput=False)
    w1 = nc.declare_dram_parameter("w1", [DA, H], dt.bfloat16, isOutput=False)
    w2 = nc.declare_dram_parameter("w2", [H, H], dt.bfloat16, isOutput=False)      # 2 x [128, 256] k-chunks stacked
    w2t = nc.declare_dram_parameter("w2t", [H, H], dt.bfloat16, isOutput=False)    # W2.T, same chunking
    w3 = nc.declare_dram_parameter("w3", [H, D], dt.bfloat16, isOutput=False)      # 2 x [128, 64]
    w3t = nc.declare_dram_parameter("w3t", [D, H], dt.bfloat16, isOutput=False)    # [64, 256]
    w1xt = nc.declare_dram_parameter("w1xt", [H, D], dt.bfloat16, isOutput=False)  # W1[:64].T, 2 x [128, 64]
    b1p = nc.declare_dram_parameter("b1p", [128, 2], dt.float32, isOutput=False)
    b2p = nc.declare_dram_parameter("b2p", [128, 2], dt.float32, isOutput=False)
    w3s = nc.declare_dram_parameter("w3s", [128, 2], dt.float32, isOutput=False)   # rowsum(W3) chunks
    idn = nc.declare_dram_parameter("idn", [128, 128], dt.bfloat16, isOutput=False)
    if with_b3:
        b3bc = nc.declare_dram_parameter("b3bc", [128, NS // 128 * D], dt.float32, isOutput=False)
    dx = nc.declare_dram_parameter("dx", [nc_samples, D], dt.float32, isOutput=True)
    # view: sample n = chunk*128 + p  ->  [p, chunk, f]
    dx_v = dx.rearrange("(c p) f -> p c f", p=128)

    with tile.TileContext(nc) as tc, ExitStack() as ctx:
        cst = ctx.enter_context(tc.tile_pool(name="cst", bufs=1))
        xp = ctx.enter_context(tc.tile_pool(name="xp", bufs=2))
        h1p = ctx.enter_context(tc.tile_pool(name="h1p", bufs=3))
        h2p = ctx.enter_context(tc.tile_pool(name="h2p", bufs=3))
        g1p = ctx.enter_context(tc.tile_pool(name="g1p", bufs=3))
        g2p = ctx.enter_context(tc.tile_pool(name="g2p", bufs=3))
        dap = ctx.enter_context(tc.tile_pool(name="dap", bufs=3))
        nrm = ctx.enter_context(tc.tile_pool(name="nrm", bufs=2))
        sml = ctx.enter_context(tc.tile_pool(name="sml", bufs=2))
        dxp = ctx.enter_context(tc.tile_pool(name="dxp", bufs=2))
        fps = ctx.enter_context(tc.tile_pool(name="fps", bufs=2, space="PSUM"))
        bps = ctx.enter_context(tc.tile_pool(name="bps", bufs=2, space="PSUM"))
        sps = ctx.enter_context(tc.tile_pool(name="sps", bufs=2, space="PSUM"))
        tps = ctx.enter_context(tc.tile_pool(name="tps", bufs=1, space="PSUM"))
        ops = ctx.enter_context(tc.tile_pool(name="ops", bufs=1, space="PSUM"))

        # ---- one-time constants ----
        w1_s = cst.tile([DA, H], dt.bfloat16, name="w1_s")
        nc.sync.dma_start(w1_s[:], w1[:])
        w2_s = [cst.tile([128, H], dt.bfloat16, name=f"w2_{k}") for k in range(2)]
        w2t_s = [cst.tile([128, H], dt.bfloat16, name=f"w2t_{k}") for k in range(2)]
        w3_s = [cst.tile([128, D], dt.bfloat16, name=f"w3_{k}") for k in range(2)]
        w1xt_s = [cst.tile([128, D], dt.bfloat16, name=f"w1xt_{k}") for k in range(2)]
        for k in range(2):
            nc.sync.dma_start(w2_s[k][:], w2[128 * k:128 * (k + 1), :])
            nc.sync.dma_start(w2t_s[k][:], w2t[128 * k:128 * (k + 1), :])
            nc.sync.dma_start(w3_s[k][:], w3[128 * k:128 * (k + 1), :])
            nc.sync.dma_start(w1xt_s[k][:], w1xt[128 * k:128 * (k + 1), :])
        w3t_s = cst.tile([D, H], dt.bfloat16, name="w3t_s")
        nc.sync.dma_start(w3t_s[:], w3t[:])
        b1_s = cst.tile([128, 2], dt.float32, name="b1_s")
        nc.sync.dma_start(b1_s[:], b1p[:])
        b2_s = cst.tile([128, 2], dt.float32, name="b2_s")
        nc.sync.dma_start(b2_s[:], b2p[:])
        w3s_s = cst.tile([128, 2], dt.float32, name="w3s_s")
        nc.sync.dma_start(w3s_s[:], w3s[:])
        idn_s = cst.tile([128, 128], dt.bfloat16, name="idn_s")
        nc.sync.dma_start(idn_s[:], idn[:])
        magic_s = cst.tile([128, 8], dt.int32, name="magic_s")
        nc.gpsimd.memset(magic_s[:], MAGIC)
        if with_b3:
            b3bc_s = cst.tile([128, NS // 128 * D], dt.float32, name="b3bc_s")
            nc.sync.dma_start(b3bc_s[:], b3bc[:])

        NCH = NS // 128   # 128-sample chunks per mega (16)
        HCH = NCH // 2    # chunks per half-mega (8)
        NT = 512

        for mg in range(n_megas):
            x_t = xp.tile([DA, NS], dt.bfloat16, tag="x", name="x_t")
            nc.sync.dma_start(x_t[:], xT[:, mg * NS:(mg + 1) * NS])

            h1_t = [h1p.tile([128, NS], dt.bfloat16, tag=f"h1_{m}", name=f"h1_{m}") for m in range(2)]
            g1_t = [g1p.tile([128, NS], dt.bfloat16, tag=f"g1_{m}", name=f"g1_{m}") for m in range(2)]
            h2_t = [h2p.tile([128, NS], dt.bfloat16, tag=f"h2_{m}", name=f"h2_{m}") for m in range(2)]
            g2_t = [g2p.tile([128, NS], dt.bfloat16, tag=f"g2_{m}", name=f"g2_{m}") for m in range(2)]
            sr = nrm.tile([D, NS], dt.bfloat16, tag="sr", name="sr")

            for hh in range(2):  # half-mega: fwd + norm + transpose
                s_ps = sps.tile([128, HCH * D], dt.float32, tag="s", name="s_ps")
                for j in (2 * hh, 2 * hh + 1):
                    sl = slice(j * NT, (j + 1) * NT)
                    for m in range(2):
                        a1 = fps.tile([128, NT], dt.float32, tag="fwd", name="a1")
                        nc.tensor.matmul(a1[:], w1_s[:, 128 * m:128 * (m + 1)],
                                         x_t[:, sl], start=True, stop=True)
                        nc.scalar.activation(h1_t[m][:, sl], a1[:], AF.Gelu,
                                             bias=b1_s[:, m:m + 1])
                        nc.scalar.activation(g1_t[m][:, sl], a1[:], AF.Derivative_Gelu,
                                             bias=b1_s[:, m:m + 1])
                    for m in range(2):
                        a2 = fps.tile([128, NT], dt.float32, tag="fwd", name="a2")
                        for k in range(2):
                            nc.tensor.matmul(a2[:], w2_s[k][:, 128 * m:128 * (m + 1)],
                                             h1_t[k][:, sl], start=(k == 0), stop=(k == 1))
                        nc.scalar.activation(h2_t[m][:, sl], a2[:], AF.Gelu,
                                             bias=b2_s[:, m:m + 1])
                        nc.scalar.activation(g2_t[m][:, sl], a2[:], AF.Derivative_Gelu,
                                             bias=b2_s[:, m:m + 1])
                    for sc in range(4):
                        ch = (j - 2 * hh) * 4 + sc
                        csl = slice(j * NT + sc * 128, j * NT + (sc + 1) * 128)
                        for k in range(2):
                            nc.tensor.matmul(s_ps[:, ch * D:(ch + 1) * D],
                                             h2_t[k][:, csl], w3_s[k][:],
                                             start=(k == 0), stop=(k == 1))

                # norm for this half: negr = -1/||s||
                if with_b3:
                    s_in = nrm.tile([128, HCH * D], dt.float32, tag="sb3", name="s_in")
                    nc.vector.tensor_tensor(
                        s_in[:], s_ps[:], b3bc_s[:, :HCH * D], ALU.add)
                else:
                    s_in = s_ps
                sq = nrm.tile([128, HCH * D], dt.float32, tag="sq", name="sq")
                nc.scalar.activation(sq[:], s_in[:], AF.Square)
                q = sml.tile([128, HCH], dt.float32, tag="q", name="q")
                nc.vector.tensor_reduce(q[:], sq[:].rearrange("p (c f) -> p c f", f=D),
                                        mybir.AxisListType.X, ALU.add)
                yi = sml.tile([128, HCH], dt.int32, tag="yi", name="yi")
                nc.vector.tensor_scalar(yi[:], q[:].bitcast(dt.int32), 1, None,
                                        ALU.logical_shift_right)
                nc.vector.tensor_tensor(yi[:], magic_s[:], yi[:], ALU.subtract)
                y = yi[:].bitcast(dt.float32)
                qh = sml.tile([128, HCH], dt.float32, tag="qh", name="qh")
                nc.vector.tensor_scalar(qh[:], q[:], 0.5, None, ALU.mult)
                pp = sml.tile([128, HCH], dt.float32, tag="pp", name="pp")
                for it in range(3):
                    nc.vector.tensor_tensor(pp[:], y, y, ALU.mult)
                    nc.vector.tensor_tensor(pp[:], pp[:], qh[:], ALU.mult)
                    nc.vector.tensor_scalar(pp[:], pp[:], -1.5, None, ALU.add)
                    nc.vector.tensor_tensor(y, y, pp[:], ALU.mult)  # y <- -y(1.5-.5qy^2)
                nrb = nrm.tile([128, HCH * D], dt.float32, tag="nrb", name="nrb")
                nc.gpsimd.tensor_copy(
                    nrb[:].rearrange("p (c f) -> p c f", f=D),
                    y.broadcast_to([128, HCH, D]))
                srt = nrm.tile([128, HCH * D], dt.bfloat16, tag="srt", name="srt")
                nc.vector.tensor_tensor(srt[:], s_in[:], nrb[:], ALU.mult)

                sr_ps = tps.tile([D, NS // 2], dt.bfloat16, tag="srps", name="sr_ps")
                for c in range(HCH):
                    nc.tensor.transpose(sr_ps[:, c * 128:(c + 1) * 128],
                                        srt[:, c * D:(c + 1) * D], idn_s[:])
                nc.vector.tensor_copy(sr[:, hh * (NS // 2):(hh + 1) * (NS // 2)], sr_ps[:])

            # ---------------- backward ----------------
            da2_t = [dap.tile([128, NS], dt.bfloat16, tag=f"da2_{m}", name=f"da2_{m}") for m in range(2)]
            da1_t = [dap.tile([128, NS], dt.bfloat16, tag=f"da1_{m}", name=f"da1_{m}") for m in range(2)]
            for j in range(NS // NT):
                sl = slice(j * NT, (j + 1) * NT)
                for m in range(2):
                    u = bps.tile([128, NT], dt.float32, tag="bwd", name="u")
                    nc.tensor.matmul(u[:], w3t_s[:, 128 * m:128 * (m + 1)], sr[:, sl],
                                     start=True, stop=True)
                    # da2 = (u + rowsum(W3)) * ga2    [dh2 = W3@(1 - s r)]
                    nc.vector.scalar_tensor_tensor(da2_t[m][:, sl], u[:],
                                                   w3s_s[:, m:m + 1], g2_t[m][:, sl],
                                                   ALU.add, ALU.mult)
                for m in range(2):
                    dh1 = bps.tile([128, NT], dt.float32, tag="bwd", name="dh1")
                    for k in range(2):
                        nc.tensor.matmul(dh1[:], w2t_s[k][:, 128 * m:128 * (m + 1)],
                                         da2_t[k][:, sl], start=(k == 0), stop=(k == 1))
                    nc.vector.tensor_tensor(da1_t[m][:, sl], dh1[:], g1_t[m][:, sl],
                                            ALU.mult)
                dxq = ops.tile([128, 4 * D], dt.float32, tag="dxq", name="dxq")
                for sc in range(4):
                    csl = slice(j * NT + sc * 128, j * NT + (sc + 1) * 128)
                    for k in range(2):
                        nc.tensor.matmul(dxq[:, sc * D:(sc + 1) * D],
                                         da1_t[k][:, csl], w1xt_s[k][:],
                                         start=(k == 0), stop=(k == 1))
                dxs = dxp.tile([128, 4 * D], dt.float32, tag="dxs", name="dxs")
                nc.vector.tensor_copy(dxs[:], dxq[:])
                nc.gpsimd.dma_start(
                    dx_v[:, mg * NCH + j * 4:mg * NCH + (j + 1) * 4, :],
                    dxs[:].rearrange("p (c f) -> p c f", f=D))

    nc.compile()
    return nc


_CACHE = {}


def _get_nc(nc_samples, with_b3):
    key = (nc_samples, with_b3)
    if key not in _CACHE:
        _CACHE[key] = build(nc_samples, with_b3)
    return _CACHE[key]


def kernel(t, x, W1, b1, W2, b2, W3, b3, c):
    t = np.asarray(t); x = np.asarray(x)
    W1 = np.asarray(W1, np.float32); b1 = np.asarray(b1, np.float32)
    W2 = np.asarray(W2, np.float32); b2 = np.asarray(b2, np.float32)
    W3 = np.asarray(W3, np.float32); b3 = np.asarray(b3, np.float32)
    N = t.shape[0]
    npc = N // NCORES
    with_b3 = bool(np.any(b3))
    nc = _get_nc(npc, with_b3)

    xT = np.empty((DA, N), dtype=BF)
    xT[:D] = x.T
    xT[D] = t
    base = dict(
        w1=W1.astype(BF),
        w2=W2.astype(BF),
        w2t=np.ascontiguousarray(W2.T).astype(BF),
        w3=W3.astype(BF),
        w3t=np.ascontiguousarray(W3.T).astype(BF),
        w1xt=np.ascontiguousarray(W1[:D].T).astype(BF),
        b1p=np.ascontiguousarray(b1.reshape(2, 128).T),
        b2p=np.ascontiguousarray(b2.reshape(2, 128).T),
        w3s=np.ascontiguousarray(W3.sum(1).astype(np.float32).reshape(2, 128).T),
        idn=np.eye(128, dtype=BF),
    )
    if with_b3:
        base["b3bc"] = np.tile(b3, (128, NS // 128)).astype(np.float32)
    in_maps = []
    for cid in range(NCORES):
        m = dict(base)
        m["xT"] = np.ascontiguousarray(xT[:, cid * npc:(cid + 1) * npc])
        in_maps.append(m)
    res = run_bass_kernel_spmd(nc, in_maps, list(range(NCORES)))
    return np.concatenate([res.results[i]["dx"] for i in range(NCORES)], axis=0)
